# revision 12
# baseline (speedup 1.0000x reference)
"""ContrastiveLoss distributed Trainium2 kernel (8 NeuronCores).

Reference math:
  t = l2norm(textual); c0 = l2norm(f0) @ t.T; c1 = l2norm(f1) @ t.T
  loss = sum(lab*(1-c) + (1-lab)*relu(c-1)) over both c / B^2

Key identity: cosine similarity is <= 1 by Cauchy-Schwarz (the EPS-clamped
denominator max(|x|,eps)*max(|t|,eps) >= |x||t| only shrinks it), so
relu(c-1) == 0 exactly for every pair, for ANY real inputs. The loss is
therefore identically
  loss = sum_ij lab[i,j] * (1 - cos(x[i], t[j])) / B^2.

Fast path (labels == I, verified exactly on host): only the diagonal
cos(x[i], t[i]) terms survive, i.e. rowwise dots. Rows are sharded across
the 8 cores (512 rows each); each core computes, fully on device:
  ssq rows of x0/x1/t (ACT Square+accum), 1/max(sqrt,eps) norms,
  raw dots x.t per row (DVE tensor_tensor_reduce), d = dot*rx*rt,
  out[p] = sum over its rows of (d0+d1).
Host: loss = (2B - sum(out)) / B^2.

General-labels fallback (not hit by the reference generator): same reduced
formula with arbitrary lab via g = lab @ t_hat, loss = sum lab - sum x_hat.g
rowwise, computed on host in f32 BLAS.
"""
import sys

if "/opt/trn_rl_repo" not in sys.path:
    sys.path.insert(0, "/opt/trn_rl_repo")

import numpy as np
import ml_dtypes

import concourse.bass as bass
import concourse.mybir as mybir
import concourse.tile as tile
import bass_rust

B, D = 4096, 1024
NCORES = 8
RPC = B // NCORES          # rows per core = 512
P = 128
OB = RPC // P              # 4 row-blocks of 128 per core
bf16 = mybir.dt.bfloat16
f32 = mybir.dt.float32
EPS = 1e-8

_CACHE = {}


def _split_waits(nc, max_waits=1):
    """This walrus build rejects >1 semaphore wait per instruction; hoist
    extras onto same-engine NOPs placed immediately before."""
    SI = bass_rust.SyncInfo
    n = 0
    for bb in nc.main_func.blocks:
        new_insts, changed = [], False
        for inst in bb.instructions:
            si = inst.sync_info
            if si is None:
                new_insts.append(inst)
                continue
            waits = list(si.on_wait)
            if len(waits) > max_waits:
                extra, keep = waits[:-max_waits], waits[-max_waits:]
                for j in range(0, len(extra), max_waits):
                    nop = mybir.InstNoOp(name=f"{inst.name}-ws{j}", ins=[], outs=[])
                    nop.engine = inst.engine
                    nop.sync_info = SI(on_wait=extra[j : j + max_waits], on_update=[])
                    nc.register_instruction(nop, overwrite=True)
                    new_insts.append(nop)
                    n += 1
                inst.sync_info = SI(on_wait=keep, on_update=list(si.on_update))
                changed = True
            new_insts.append(inst)
        if changed:
            bb.instructions = new_insts
    return n


def _build(reps=1):
    """reps>1 repeats the whole computation in one NEFF (used only by the
    throughput benchmark to amortize per-dispatch overhead; production=1)."""
    nc = bass.Bass("TRN2", target_bir_lowering=False, debug=False,
                   num_devices=NCORES)
    A = mybir.AluOpType
    AF = mybir.ActivationFunctionType

    x0 = nc.dram_tensor("x0", [RPC, D], bf16, kind="ExternalInput").ap()
    x1 = nc.dram_tensor("x1", [RPC, D], bf16, kind="ExternalInput").ap()
    ts = nc.dram_tensor("ts", [RPC, D], bf16, kind="ExternalInput").ap()
    out = nc.dram_tensor("out", [P, 1], f32, kind="ExternalOutput").ap()

    # engine assignment per (tensor, block) pass, balanced by the cost
    # model's per-op rates (DVE 1.13us/block, ACT 1.23us/block) so both
    # engines stay at/under the ~12.5us input-DMA time. ACT only does
    # squares (single-operand); dots need a two-tensor op (DVE). The Pool
    # engine can't run TensorScalar on this walrus build.
    # ACT: 10 squares (12.3us), DVE: 2 squares + 8 dots (11.3us) -- both
    # just under the steady-state DMA time so neither engine is the long
    # pole; ts's late blocks go to DVE so ACT's queue drains early.
    sq_eng = {(n, o): ("vector" if n == "ts" and o >= 2 else "scalar")
              for n in ("ts", "x0", "x1") for o in range(4)}
    dot_eng = {(n, o): "vector" for n in ("x0", "x1") for o in range(4)}

    with tile.TileContext(nc) as tc:
        with (
            tc.tile_pool(name="big", bufs=3 if reps > 1 else 1) as big,
            tc.tile_pool(name="work", bufs=8) as work,
            tc.tile_pool(name="small", bufs=3 if reps > 1 else 1) as small,
        ):
            for _ in range(reps):
                # resident loads; "(p o)" row mapping gives each partition
                # contiguous rows (the row->partition permutation is
                # irrelevant: every consumer reduces over all rows). Two
                # chunks per tensor so compute overlaps the tail of each
                # load without per-DMA overhead dominating.
                sb = {}
                for name, src in (("ts", ts), ("x0", x0), ("x1", x1)):
                    t_ = big.tile([P, OB, D], bf16, tag=name)
                    re = src.rearrange("(p o) d -> p o d", o=OB)
                    # 2 chunks per tensor: each is 2 consecutive rows per
                    # partition = 4KB-contiguous descriptors, and half the
                    # DMA-completion semaphores vs per-block loads
                    nc.sync.dma_start(t_[:, : OB // 2], re[:, : OB // 2])
                    nc.sync.dma_start(t_[:, OB // 2 :], re[:, OB // 2 :])
                    sb[name] = t_

                # ---- row sums of squares -> 1/max(sqrt(ssq), eps)
                rnorm, draw = {}, {}
                ssqs = {}
                for name in ("ts", "x0", "x1"):
                    ssq = small.tile([P, OB], f32, tag=f"ssq_{name}")
                    for o in range(OB):
                        scr = work.tile([P, D], bf16, tag="scr")
                        eng = getattr(nc, sq_eng[(name, o)])
                        if sq_eng[(name, o)] == "scalar":
                            eng.activation(scr[:], sb[name][:, o], AF.Square,
                                           accum_out=ssq[:, o : o + 1])
                        else:
                            eng.scalar_tensor_tensor(
                                out=scr[:], in0=sb[name][:, o], scalar=1.0,
                                in1=sb[name][:, o], op0=A.mult, op1=A.mult,
                                accum_out=ssq[:, o : o + 1])
                    ssqs[name] = ssq

                # ---- raw rowwise dots x.t (interleaved with squares by
                # the tile scheduler; engines per the table above)
                for name in ("x0", "x1"):
                    acc = small.tile([P, OB], f32, tag=f"draw_{name}")
                    for o in range(OB):
                        scr = work.tile([P, D], bf16, tag="scr")
                        eng = getattr(nc, dot_eng[(name, o)])
                        eng.scalar_tensor_tensor(
                            out=scr[:], in0=sb[name][:, o], scalar=1.0,
                            in1=sb["ts"][:, o], op0=A.mult, op1=A.mult,
                            accum_out=acc[:, o : o + 1])
                    draw[name] = acc

                for name in ("ts", "x0", "x1"):
                    r = small.tile([P, OB], f32, tag=f"rn_{name}")
                    nc.scalar.sqrt(r[:], ssqs[name][:])
                    nc.vector.tensor_scalar(r[:], r[:], EPS, None, A.max)
                    nc.vector.reciprocal(r[:], r[:])
                    rnorm[name] = r

                # ---- d = draw * rx * rt ; out[p] = sum_o d0 + d1
                s01 = small.tile([P, OB], f32, tag="s01")
                nc.vector.tensor_tensor(s01[:], draw["x0"][:], rnorm["x0"][:],
                                        A.mult)
                d1s = small.tile([P, OB], f32, tag="d1s")
                nc.vector.tensor_tensor(d1s[:], draw["x1"][:], rnorm["x1"][:],
                                        A.mult)
                nc.vector.tensor_tensor(s01[:], s01[:], d1s[:], A.add)
                nc.vector.tensor_tensor(s01[:], s01[:], rnorm["ts"][:], A.mult)
                tot = small.tile([P, 1], f32, tag="tot")
                nc.vector.tensor_reduce(tot[:], s01[:], mybir.AxisListType.X,
                                        A.add)
                nc.sync.dma_start(out, tot[:])

    _split_waits(nc, max_waits=1)
    return nc


def _get_nc():
    if "nc" not in _CACHE:
        _CACHE["nc"] = _build()
    return _CACHE["nc"]


def _get_executor(key="exec", nc=None):
    """Build (once per key) a jitted shard_map executor for the NEFF,
    mirroring concourse.bass2jax.run_bass_via_pjrt but cached so repeat
    kernel() calls don't retrace/recompile."""
    if key in _CACHE:
        return _CACHE[key]
    import jax
    from jax.sharding import Mesh, PartitionSpec, NamedSharding
    from jax.experimental.shard_map import shard_map
    from concourse.bass2jax import (
        _bass_exec_p, partition_id_tensor, install_neuronx_cc_hook)

    if nc is None:
        nc = _get_nc()
    install_neuronx_cc_hook()
    partition_name = nc.partition_id_tensor.name if nc.partition_id_tensor else None
    in_names, out_names, out_avals, zero_outs = [], [], [], []
    for alloc in nc.m.functions[0].allocations:
        if not isinstance(alloc, mybir.MemoryLocationSet):
            continue
        name = alloc.memorylocations[0].name
        if alloc.kind == "ExternalInput":
            if name != partition_name:
                in_names.append(name)
        elif alloc.kind == "ExternalOutput":
            shape = tuple(alloc.tensor_shape)
            dtype = mybir.dt.np(alloc.dtype)
            out_names.append(name)
            out_avals.append(jax.core.ShapedArray(shape, dtype))
            zero_outs.append(np.zeros(shape, dtype))
    n_params = len(in_names)
    n_outs = len(out_avals)
    all_in_names = list(in_names) + out_names
    if partition_name is not None:
        all_in_names.append(partition_name)

    def _body(*args):
        operands = list(args)
        if partition_name is not None:
            operands.append(partition_id_tensor())
        outs = _bass_exec_p.bind(
            *operands, out_avals=tuple(out_avals), in_names=tuple(all_in_names),
            out_names=tuple(out_names), lowering_input_output_aliases=(),
            sim_require_finite=True, sim_require_nnan=True, nc=nc)
        return tuple(outs)

    devices = jax.devices()[:NCORES]
    mesh = Mesh(np.asarray(devices), ("core",))
    in_specs = (PartitionSpec("core"),) * (n_params + n_outs)
    out_specs = (PartitionSpec("core"),) * len(out_names)
    sharded = jax.jit(
        shard_map(_body, mesh=mesh, in_specs=in_specs, out_specs=out_specs,
                  check_rep=False),
        donate_argnums=tuple(range(n_params, n_params + n_outs)),
        keep_unused=True)
    sh = NamedSharding(mesh, PartitionSpec("core"))
    zshapes = [(NCORES * z.shape[0], *z.shape[1:]) for z in zero_outs]
    zdtypes = [z.dtype for z in zero_outs]
    _CACHE[key] = (sharded, in_names, out_names, zshapes, zdtypes, sh)
    return _CACHE[key]


def _labels_are_identity(lb: np.ndarray) -> bool:
    if lb.shape != (B, B):
        return False
    d = lb.diagonal()
    if not (d == 1.0).all():
        return False
    return float(lb.sum(dtype=np.float64)) == float(B)


def _run_device(f0b, f1b, tb):
    """Run the NEFF on the 8 cores with row-sharded bf16 inputs; returns
    the per-core [128,1] partial sums stacked to [8,128]."""
    import jax
    sharded, in_names, out_names, zshapes, zdtypes, sh = _get_executor()
    by_name = {"x0": f0b, "x1": f1b, "ts": tb}
    dev_in = [jax.device_put(np.ascontiguousarray(by_name[nm]), sh)
              for nm in in_names]
    zs = [jax.device_put(np.zeros(s, d), sh) for s, d in zip(zshapes, zdtypes)]
    outs = sharded(*dev_in, *zs)
    return np.asarray(outs[0]).reshape(NCORES, P)


def _fallback_general(f0, f1, t, lb):
    """Arbitrary-labels path (host f32 BLAS). loss = sum lab (1-cos) / B^2."""
    def l2n(x):
        n = np.sqrt((x * x).sum(axis=-1, keepdims=True))
        return x / np.maximum(n, EPS)
    th = l2n(t)
    g = lb @ th                                   # [B, D]
    s = (l2n(f0) * g).sum() + (l2n(f1) * g).sum()
    return np.asarray((lb.sum(dtype=np.float64) * 2.0 - s) / (B * B),
                      dtype=np.float32)


def kernel(fc_feats_0, fc_feats_1, textual_features, labels):
    f0 = np.asarray(fc_feats_0, dtype=np.float32)
    f1 = np.asarray(fc_feats_1, dtype=np.float32)
    t = np.asarray(textual_features, dtype=np.float32)
    lb = np.asarray(labels, dtype=np.float32)

    if not _labels_are_identity(lb):
        return _fallback_general(f0, f1, t, lb)

    bf = ml_dtypes.bfloat16
    parts = _run_device(f0.astype(bf), f1.astype(bf), t.astype(bf))
    total = parts.sum(dtype=np.float64)
    return np.asarray((2.0 * B - total) / (B * B), dtype=np.float32)
